# revision 9
# baseline (speedup 1.0000x reference)
"""TopoEncoder Trainium2 kernel (8 NeuronCores, data-parallel over batch).

Pipeline per core (64 samples):
  1. DMA x-shard (two HWDGE queues, 6.4KB descriptors), mean over T
     (DVE add-tree + PE pair-matrix matmul folding the two T-halves)
  2. pairwise channel-L2 distance matrix d [64,25,25]
  3. local min/max -> partition_all_reduce -> cross-core AllReduce(max)
     of (max, -min)  [overlaps the closure]
  4. Floyd-Warshall min-max closure M (25 steps) -> MST mask = (M >= d)
     (0-dim persistence deaths = MST edge weight multiset; downstream
      structure-element sum is permutation-invariant, so order is free)
  5. top-24 extraction of masked upper-tri values (max8 + match_replace)
  6. normalize deaths with global min/max, structure-element layer:
     out[b,e] = exp(-(s1*c1)^2) * sum_p exp(-s2^2 (dn_p - c2)^2)

All constant broadcasts across partitions are built with K=1 PE matmuls
(ones[1,B] (x) row) — partition-broadcast DMAs on SWDGE are ~7x slower
per packet and starve the big x transfer.
"""

from contextlib import ExitStack

import numpy as np

import bass_rust
import concourse.bass as bass
import concourse.tile as tile
from concourse import mybir
from concourse.bass_utils import run_bass_kernel_spmd

N_CORES = 8
B = 64          # samples per core
C, T, V, E = 3, 128, 25, 64
VV = V * V
NT = V - 1      # deaths per sample (24)
DT = mybir.dt.float32


def _split_excess_waits(nc, cap=1):
    """The walrus build in this env rejects instructions carrying more than
    ~2 semaphore-wait commands. Move excess waits onto same-engine NOPs
    inserted immediately before the offending instruction."""
    n_split = 0
    for bb in nc.main_func.blocks:
        insts = bb.instructions
        i = 0
        while i < len(insts):
            ins = insts[i]
            si = ins.sync_info
            waits = list(si.on_wait) if si and si.on_wait else []
            if len(waits) > cap:
                extra, keep = waits[:-cap], waits[-cap:]
                ins.sync_info = mybir.SyncInfo(
                    on_wait=keep, on_update=list(si.on_update or [])
                )
                for j, w in enumerate(extra):
                    nop = bass_rust.InstNoOp(
                        name=f"I-wsplit-{n_split}-{j}",
                        engine=ins.engine,
                        sync_info=mybir.SyncInfo(on_wait=[w], on_update=[]),
                    )
                    insts.insert(i, nop)
                    i += 1
                n_split += 1
            i += 1
    return n_split


def _build_program():
    A = mybir.AluOpType
    ACT = mybir.ActivationFunctionType
    nc = bass.Bass("TRN2", debug=False, num_devices=N_CORES)

    x_in = nc.dram_tensor("x", [B, C, T, V], DT, kind="ExternalInput").ap()
    csT_in = nc.dram_tensor("csT", [1, 4 * E], DT, kind="ExternalInput").ap()
    pm_in = nc.dram_tensor("pm", [128, B], DT, kind="ExternalInput").ap()
    ut_in = nc.dram_tensor("ut", [1, VV], DT, kind="ExternalInput").ap()
    id_in = nc.dram_tensor("id64", [B, B], DT, kind="ExternalInput").ap()
    out_d = nc.dram_tensor("out", [B, E], DT, kind="ExternalOutput").ap()

    with tile.TileContext(nc, num_cores=N_CORES) as tc, ExitStack() as ctx:
        sb = ctx.enter_context(tc.tile_pool(name="sb", bufs=1))
        work = ctx.enter_context(tc.tile_pool(name="work", bufs=2))
        psum = ctx.enter_context(tc.tile_pool(name="psum", bufs=1, space="PSUM"))
        dram = ctx.enter_context(tc.tile_pool(name="dram", bufs=1, space="DRAM"))

        # ---- x DMA first: partition p = t2*64 + b, free = (c, t64, v) ----
        # two t64-half tiles so the add-tree overlaps the second half's DMA;
        # both HWDGE queues used; 800-elem contiguous runs
        xa = sb.tile([128, C, T // 4, V], DT)
        xb = sb.tile([128, C, T // 4, V], DT)
        nc.sync.dma_start(xa[0:B], x_in[:, :, 0:32, :])
        nc.scalar.dma_start(xa[B:128], x_in[:, :, 64:96, :])
        nc.sync.dma_start(xb[0:B], x_in[:, :, 32:64, :])
        nc.scalar.dma_start(xb[B:128], x_in[:, :, 96:128, :])

        # ---- small constant loads (HWDGE, few descriptors) ----
        pm_t = sb.tile([128, B], DT)
        nc.sync.dma_start(pm_t[:], pm_in[:])
        cst = sb.tile([1, 4 * E], DT)
        nc.scalar.dma_start(cst[:], csT_in[:])
        utrow = sb.tile([1, VV], DT)
        nc.scalar.dma_start(utrow[:], ut_in[:])
        id64 = sb.tile([B, B], DT)
        nc.sync.dma_start(id64[:], id_in[:])
        ones1 = sb.tile([1, B], DT)
        nc.vector.memset(ones1[:], 1.0)
        eps = sb.tile([128, 1], DT)
        nc.vector.memset(eps[:], 1e-12)

        # ---- PE partition-broadcasts: ones[1,B].T @ row[1,N] = [B, N] ----
        utb = psum.tile([B, VV], DT)
        nc.tensor.matmul(out=utb[:, 0:512], lhsT=ones1[:], rhs=utrow[:, 0:512],
                         start=True, stop=True)
        nc.tensor.matmul(out=utb[:, 512:VV], lhsT=ones1[:], rhs=utrow[:, 512:VV],
                         start=True, stop=True)
        prm = psum.tile([B, 4, E], DT)
        nc.tensor.matmul(out=prm[:], lhsT=ones1[:], rhs=cst[:], start=True, stop=True)
        c1p, c2p, s1p, s2p = (prm[:, 0, :], prm[:, 1, :], prm[:, 2, :], prm[:, 3, :])

        # derived parameter tiles (SBUF): u = s2^2, Ab = exp(-(s1*c1)^2)
        s2sb = sb.tile([B, E], DT)
        nc.vector.tensor_copy(s2sb[:], s2p)
        s1sb = sb.tile([B, E], DT)
        nc.vector.tensor_copy(s1sb[:], s1p)
        ub = sb.tile([B, E], DT)
        nc.vector.tensor_tensor(out=ub[:], in0=s2sb[:], in1=s2sb[:], op=A.mult)
        m1t = sb.tile([B, E], DT)
        nc.vector.tensor_tensor(out=m1t[:], in0=s1sb[:], in1=c1p, op=A.mult)
        nc.scalar.square(m1t[:], m1t[:])
        Ab = sb.tile([B, E], DT)
        nc.scalar.activation(Ab[:], m1t[:], ACT.Exp, bias=0.0, scale=-1.0)
        c2b = sb.tile([B, E], DT)
        nc.vector.tensor_copy(c2b[:], c2p)

        # ---- mean over T: in-place add trees, then PE pair-matrix fold ----
        for xh in (xa, xb):
            for w in (16, 8, 4, 2, 1):
                nc.vector.tensor_tensor(
                    out=xh[:, :, 0:w, :],
                    in0=xh[:, :, 0:w, :],
                    in1=xh[:, :, w : 2 * w, :],
                    op=A.add,
                )
        nc.vector.tensor_tensor(
            out=xa[:, :, 0:1, :], in0=xa[:, :, 0:1, :], in1=xb[:, :, 0:1, :], op=A.add
        )
        ps_xm = psum.tile([B, C, V], DT)
        nc.tensor.matmul(out=ps_xm[:], lhsT=pm_t[:], rhs=xa[:, :, 0, :],
                         start=True, stop=True)
        xm = sb.tile([B, C, V], DT)
        nc.vector.tensor_copy(xm[:], ps_xm[:])

        # ---- distance matrix ----
        df = sb.tile([B, C, V, V], DT)
        nc.vector.tensor_tensor(
            out=df[:],
            in0=xm.unsqueeze(-1).broadcast_to([B, C, V, V]),
            in1=xm.unsqueeze(2).broadcast_to([B, C, V, V]),
            op=A.subtract,
        )
        nc.scalar.square(df[:], df[:])
        d2 = sb.tile([B, VV], DT)
        d23 = d2.rearrange("p (i j) -> p i j", i=V)
        nc.vector.tensor_tensor(out=d23[:], in0=df[:, 0], in1=df[:, 1], op=A.add)
        nc.vector.tensor_tensor(out=d23[:], in0=d23[:], in1=df[:, 2], op=A.add)
        dmat = sb.tile([B, VV], DT)
        nc.scalar.activation(dmat[:], d2[:], ACT.Sqrt, bias=eps[0:B, 0:1], scale=1.0)

        # ---- local min/max -> DRAM-bounce partition transpose ----
        lmm = sb.tile([B, 2], DT)
        nc.vector.tensor_reduce(out=lmm[:, 0:1], in_=dmat[:],
                                axis=mybir.AxisListType.X, op=A.max)
        nc.vector.tensor_reduce(out=lmm[:, 1:2], in_=dmat[:],
                                axis=mybir.AxisListType.X, op=A.min)
        nc.vector.tensor_scalar_mul(lmm[:, 1:2], lmm[:, 1:2], -1.0)
        lmmT = psum.tile([2, B], DT)
        nc.tensor.matmul(out=lmmT[:], lhsT=lmm[:], rhs=id64[:], start=True, stop=True)

        # ---- premasked values (overlaps FW head) ----
        dut = sb.tile([B, VV], DT)
        nc.vector.tensor_tensor(out=dut[:], in0=dmat[:], in1=utb[:], op=A.mult)

        # ---- Floyd-Warshall min-max closure (in place) ----
        # the global-minmax reduce + collective chain is spliced in after
        # step 2 so the bounce-DMA latency hides behind the closure
        M = sb.tile([B, VV], DT)
        M3 = M.rearrange("p (i j) -> p i j", i=V)
        dm3 = dmat.rearrange("p (i j) -> p i j", i=V)
        fwt = sb.tile([B, V, V], DT)
        gmr = sb.tile([2, 1], DT)
        cin = dram.tile([2, 1], DT)
        cout = dram.tile([2, 1], DT)
        gsb = sb.tile([1, 2], DT)
        for k in range(V):
            src = dm3 if k == 0 else M3
            nc.vector.tensor_tensor(
                out=fwt[:],
                in0=src[:, :, k : k + 1].broadcast_to([B, V, V]),
                in1=src[:, k : k + 1, :].broadcast_to([B, V, V]),
                op=A.max,
            )
            nc.vector.tensor_tensor(out=M3[:], in0=src[:], in1=fwt[:], op=A.min)
            if k == 6:
                nc.vector.tensor_reduce(out=gmr[:], in_=lmmT[:],
                                        axis=mybir.AxisListType.X, op=A.max)
            if k == 7:
                nc.scalar.dma_start(cin[:], gmr[:])
                nc.gpsimd.collective_compute(
                    "AllReduce", A.max, replica_groups=[list(range(N_CORES))],
                    ins=[cin.opt()], outs=[cout.opt()],
                )
                nc.scalar.dma_start(gsb[:], cout.transpose([1, 0]))
                gbp = psum.tile([B, 2], DT)
                nc.tensor.matmul(out=gbp[:], lhsT=ones1[:], rhs=gsb[:],
                                 start=True, stop=True)

        # ---- MST mask + masked upper-tri values ----
        mk = sb.tile([B, VV], DT)
        nc.vector.tensor_tensor(out=mk[:], in0=M[:], in1=dmat[:], op=A.is_ge)
        val = sb.tile([B, VV], DT)
        nc.vector.tensor_tensor(out=val[:], in0=mk[:], in1=dut[:], op=A.mult)

        # ---- extract 24 MST weights: 3 rounds of top-8 + match_replace ----
        deaths = sb.tile([B, NT], DT)
        mr1 = sb.tile([B, VV], DT)
        mr2 = sb.tile([B, VV], DT)
        nc.vector.max(deaths[:, 0:8], val[:])
        nc.vector.match_replace(mr1[:], deaths[:, 0:8], val[:], 0.0)
        nc.vector.max(deaths[:, 8:16], mr1[:])
        nc.vector.match_replace(mr2[:], deaths[:, 8:16], mr1[:], 0.0)
        nc.vector.max(deaths[:, 16:24], mr2[:])

        # ---- normalize deaths (global min/max ready long before this) ----
        gb = sb.tile([B, 2], DT)
        nc.vector.tensor_copy(gb[:], gbp[:])
        rngb = sb.tile([B, 1], DT)
        nc.vector.tensor_tensor(out=rngb[:], in0=gb[:, 0:1], in1=gb[:, 1:2], op=A.add)
        invb = sb.tile([B, 1], DT)
        nc.vector.reciprocal(invb[:], rngb[:])
        gminb = sb.tile([B, 1], DT)
        nc.vector.tensor_scalar_mul(gminb[:], gb[:, 1:2], -1.0)
        dn = sb.tile([B, NT], DT)
        nc.vector.tensor_scalar(
            out=dn[:], in0=deaths[:], scalar1=gminb[:, 0:1], scalar2=invb[:, 0:1],
            op0=A.subtract, op1=A.mult,
        )

        # ---- structure element layer ----
        S = sb.tile([B, E], DT)
        ECH = 32
        for ch in range(E // ECH):
            e0 = ch * ECH
            t1 = work.tile([B, ECH, NT], DT, tag="t1")
            nc.vector.tensor_tensor(
                out=t1[:],
                in0=dn.unsqueeze(1).broadcast_to([B, ECH, NT]),
                in1=c2b[:, e0 : e0 + ECH].unsqueeze(-1).broadcast_to([B, ECH, NT]),
                op=A.subtract,
            )
            nc.scalar.square(t1[:], t1[:])
            nc.vector.tensor_tensor(
                out=t1[:],
                in0=t1[:],
                in1=ub[:, e0 : e0 + ECH].unsqueeze(-1).broadcast_to([B, ECH, NT]),
                op=A.mult,
            )
            fexp = work.tile([B, ECH, NT], DT, tag="fexp")
            nc.scalar.activation(fexp[:], t1[:], ACT.Exp, bias=0.0, scale=-1.0)
            nc.vector.tensor_reduce(
                out=S[:, e0 : e0 + ECH], in_=fexp[:], axis=mybir.AxisListType.X,
                op=A.add,
            )
        outt = sb.tile([B, E], DT)
        nc.vector.tensor_tensor(out=outt[:], in0=S[:], in1=Ab[:], op=A.mult)
        nc.sync.dma_start(out_d[:], outt[:])

    _split_excess_waits(nc)
    return nc


_CACHE = {}


def _consts():
    # pair matrix: adds partition rows b and b+64 (the two T-halves) and
    # applies the 1/T mean scale
    pairmat = np.zeros((128, B), dtype=np.float32)
    for p in range(128):
        pairmat[p, p % B] = 1.0 / T
    ut = np.triu(np.ones((V, V), dtype=np.float32), k=1).reshape(1, VV)
    return pairmat, np.ascontiguousarray(ut), np.eye(B, dtype=np.float32)


def _get_program():
    if "nc" not in _CACHE:
        _CACHE["nc"] = _build_program()
    return _CACHE["nc"]


def _run(x, centres, sharpness, **run_kwargs):
    nc = _get_program()
    xf = np.ascontiguousarray(x.reshape(-1, C, T, V)).astype(np.float32, copy=False)
    n_total = xf.shape[0]
    assert n_total == N_CORES * B, xf.shape
    csT = np.ascontiguousarray(
        np.stack(
            [centres[:, 0], centres[:, 1], sharpness[:, 0], sharpness[:, 1]], axis=0
        ).astype(np.float32).reshape(1, 4 * E)
    )
    pairmat, ut, id64 = _consts()
    in_maps = [
        {
            "x": np.ascontiguousarray(xf[i * B : (i + 1) * B]),
            "csT": csT,
            "pm": pairmat,
            "ut": ut,
            "id64": id64,
        }
        for i in range(N_CORES)
    ]
    res = run_bass_kernel_spmd(nc, in_maps, list(range(N_CORES)), **run_kwargs)
    out = np.concatenate([res.results[i]["out"] for i in range(N_CORES)], axis=0)
    return out, res


def kernel(x, centres, sharpness):
    out, _ = _run(np.asarray(x), np.asarray(centres), np.asarray(sharpness))
    return out
